# revision 38
# baseline (speedup 1.0000x reference)
"""Trainium2 Bass kernel: out = relu(L0@(X@W0) + L1@(X@W1) + L2@(X@W2) + bias).

Shapes: X [8192, 32], Lk [8192, 8192], Wk [32, 32], bias [32] (f32 inputs).

Strategy (8 NeuronCores, node-dim sharding per the sharding hint):
  - Each core owns a 1024-row block of the output: C_c = sum_g Lg[rows_c] @ (X@Wg).
  - Computed transposed on-chip:  C_c.T = sum_g (X@Wg).T @ Lg[rows_c].T
    so the big Lg data is the *moving* PE operand (streamed through the
    systolic array) and the tiny Y = X@Wg tiles are the stationary weights.
  - The PE contracts over the partition dim, so Lg needs its column index on
    partitions.  f32 DMA-transpose doesn't exist on TRN2; the transpose is
    done host-side as part of sharding: each core receives Lg[rows_c].T
    contiguous, so every device DMA is a pure line-rate stream and the kernel
    is HBM-bandwidth bound (the problem's target regime).
  - L is streamed as fp8 e3m4 (host-cast): quarter of the f32 HBM traffic.
    With 4 mantissa bits the measured end-to-end max-abs/scale error is
    ~1.3e-2 (vs ~3.2e-4 for fp16 at 2x the bytes) -- inside the 2e-2 gate.
    At 1 B/elem the PE stream (1 col/cycle, 196608 cols, ~82us warm) becomes
    the critical engine instead of HBM (~75us), so the kernel is PE-cadence
    bound.  The stationary Y tiles stay fp16 (mixed-dtype matmul; PE
    upconverts each operand to FP22 internally).  Set L_DTYPE below to trade
    speed vs precision.
  - Phase A computes Y = X@Wg for all g on-device in one batched pass:
    X.T is packed [128, n/4] (4 row-groups) and multiplied against all three
    W's concurrently via tile_position row-packing into 4 PSUM banks.
  - Main loop: per m-chunk of 512 nodes one PSUM bank accumulates all
    3*64 = 192 matmuls (graphs x k-tiles).  L tiles are fetched 16 k-tiles
    at a time (4MB DMAs split across both HWDGE rings); each partition reads
    a fully contiguous run, with the induced k-permutation mirrored in the
    host-side packing of X.T so lhsT/rhs agree.  Epilogue: ScalarE
    relu(acc + bias) -> SBUF -> DMA.  Output per core is C_c.T [32, 1024];
    the host transposes back while unsharding.
"""

import ml_dtypes
import numpy as np

import concourse.bacc as bacc
import concourse.mybir as mybir
import concourse.tile as tile
from concourse.bass_utils import run_bass_kernel_spmd

N = 8192
C = 32
N_CORES = 8
ROWS = N // N_CORES  # 1024

P = 128          # SBUF partitions / PE contraction tile
R = 4            # X.T row-group packing factor (phase A)
MM_N = 512       # moving-operand free dim (one fp32 PSUM bank)
T_PACK = 16      # k-tiles per L DMA
LT_BUFS = 6      # L-tile prefetch depth
# L-operand dtype for the big streamed matmuls:
#   "fp8e3": quarter HBM traffic, 4-bit mantissa (~1.3e-2 rel err)
#   "fp16": half HBM traffic, ~11-bit operand mantissa (~3.2e-4 rel err)
#   "f32r": full traffic, TF32-like PE rounding (~2.2e-4 rel err)
#   "fp32": exact (~1e-6 rel err), 4 PE cycles/row
L_DTYPE = "fp8e3"
# Stationary-operand (Y = X@W) dtype; fp16 keeps the Y quantization error
# negligible next to the fp8 L stream.
Y_DTYPE = "fp16"
CONTIG = True    # partition reads t_pack consecutive L.T rows per DMA
USE_F32R = True  # phase-A (X@W) matmuls use fp16 (fp32 when False)
N_WARM = 6       # dummy warm-up matmuls (HAM clock-gate release)


def build_nc(n=N, rows=ROWS, c=C, t_pack=T_PACK, lt_bufs=LT_BUFS,
             use_f32r=USE_F32R, l_dtype=None, contig=None, hoist=True,
             tail_split=True, n_warm=14, debug=False):
    if contig is None:
        contig = CONTIG
    if l_dtype is None:
        l_dtype = L_DTYPE
    f32 = mybir.dt.float32
    # dtype of the streamed L operand
    ldt = {"fp8e3": mybir.dt.float8e3, "fp16": mybir.dt.float16,
           "f32r": mybir.dt.float32r, "fp32": f32}[l_dtype]
    # dtype of the stationary Y tiles (must equal ldt for the fp32 paths;
    # mixed fp16 x fp8e3 is fine -- the PE upconverts operands separately)
    ydt = mybir.dt.float16 if l_dtype in ("fp16", "fp8e3") else ldt
    # phase-A (X@W) operand dtype.  fp16: the 96-col moving operand runs at
    # 1 cyc/col (f32r under 256 cols is 4 cyc/col) and LDWEIGHTS gets FWL.
    fmm = mybir.dt.float16 if (use_f32r and l_dtype != "fp32") else f32
    kt_total = n // P            # k-tiles per graph
    ni = kt_total // t_pack      # DMA iterations per graph
    mc_cnt = (rows + MM_N - 1) // MM_N
    J = kt_total // R            # inner k-tile groups for phase A packing
    c3 = 3 * c

    nc = bacc.Bacc("TRN2", target_bir_lowering=False, debug=debug)

    XT4 = nc.dram_tensor("XT4", [P, n // R], fmm, kind="ExternalInput")
    Wc4 = nc.dram_tensor("Wstack", [P, R * c3], fmm, kind="ExternalInput")
    B = nc.dram_tensor("bias", [c], f32, kind="ExternalInput")
    LT = [nc.dram_tensor(f"L{g}T", [n, rows], ldt, kind="ExternalInput")
          for g in range(3)]
    OUT = nc.dram_tensor("out", [c, rows], f32, kind="ExternalOutput")

    with tile.TileContext(nc) as tc:
        with (
            tc.tile_pool(name="const", bufs=1) as cpool,
            tc.tile_pool(name="ypool", bufs=1) as ypool,
            tc.tile_pool(name="lpool", bufs=lt_bufs) as lpool,
            tc.tile_pool(name="opool", bufs=1) as opool,
            tc.tile_pool(name="apsum", bufs=1, space="PSUM") as apsum,
            tc.tile_pool(name="mpsum", bufs=1, space="PSUM") as mpsum,
            tc.tile_pool(name="wpsum", bufs=1, space="PSUM") as wpsum,
        ):
            xt4 = cpool.tile([P, n // R], fmm)
            wc4 = cpool.tile([P, R * c3], fmm)
            bs = cpool.tile([c, 1], f32)

            # Per-ring contiguous runs >16KB/partition produced garbage on HW
            # (observed with t_pack=16 at 4-byte dtype); guard the validated
            # envelope.
            esize = mybir.dt.size(ldt)
            if contig:
                assert (t_pack // 2) * rows * esize <= 16384, \
                    "per-ring per-partition run exceeds validated 16KB"

            def lt_dma(tile_, view, tcnt):
                # Split across both HWDGE rings; each ring streams half.
                tv = tile_[:].rearrange("p (t m) -> p t m", t=tcnt)
                th = tcnt // 2
                nc.sync.dma_start(tv[:, :th], view[:, :th])
                nc.scalar.dma_start(tv[:, th:], view[:, th:])

            # HAM warm-up: the PE clock gate releases only after ~3.4us of
            # sustained matmul activity; without this the first ~25us of real
            # matmuls run at 1.2 GHz.  A burst of dummy matmuls on a zeroed
            # tile (no data dependencies) fills the DMA-bound startup window
            # and flips the gate so phase A and the main stream run at
            # 2.4 GHz from their first instruction.
            if n_warm:
                zeros = cpool.tile([P, MM_N], mybir.dt.float16)
                nc.vector.memset(zeros[:], 0.0)
                warm = wpsum.tile([P, MM_N], f32)
                # 1-col stationary: no LDWEIGHTS cost, each dummy is just the
                # 512-cycle moving stream.  They bridge PE-idle time between
                # engine init (~8us) and the consts' arrival (~10us).
                for i in range(n_warm):
                    nc.tensor.matmul(warm[:1, :], zeros[:, :1], zeros[:],
                                     start=(i == 0), stop=(i == n_warm - 1))

            # Consts first on both HWDGE rings: they are idle and
            # low-latency at kernel start, and phase A (the PE critical
            # path's head) needs them ~6us before the L stream is consumed.
            h4 = (n // R) // 2
            nc.sync.dma_start(xt4[:, :h4], XT4[:, :h4])
            nc.scalar.dma_start(xt4[:, h4:], XT4[:, h4:])
            nc.sync.dma_start(wc4[:], Wc4[:])
            nc.scalar.dma_start(bs[:], B[:][:, None])

            # First L tile: split into 4 sub-DMAs so the first mm_block can
            # start as soon as ~1/4 of the tile has landed.
            lt_pat = "(i p t) m -> i p t m" if contig else "(i t p) m -> i p t m"
            lv0 = LT[0][:].rearrange(lt_pat, t=t_pack, p=P)
            head_subs = None
            if hoist:
                sub = max(t_pack // 4, 1)
                lv00 = lv0[0].rearrange("p (u t) m -> u p t m", u=t_pack // sub)
                head_subs = []
                for u in range(t_pack // sub):
                    st = lpool.tile([P, sub * rows], ldt, tag="lt",
                                    name="lt_head")
                    lt_dma(st, lv00[u], sub)
                    head_subs.append((st, sub))

            # Phase A: Y_g = X @ Wg for all g at once, R row-groups per MM.
            # The stationary is the full [128, 128] xt4 column block (128
            # nodes x 4 row-groups of c_in); the moving operand is a
            # block-diagonal W stack [128, R*3c] (zeros off-block), so one
            # matmul emits [128 nodes, R*3c] = ys columns for the R k-tiles
            # kt = R*j..R*j+R-1 in their final layout.  The host packs XT4
            # and the L k-permutation so main-loop k-tile kt consumes
            # exactly phase-A output kt, in production order.
            ys = ypool.tile([P, kt_total * c3], ydt)
            for j in range(J):
                pa = apsum.tile([P, R * c3], f32, tag=f"pa{j % 4}",
                                name=f"pa{j % 4}")
                nc.tensor.matmul(
                    pa[:], xt4[:, j * P:(j + 1) * P], wc4[:],
                    start=True, stop=True)
                # PSUM->SBUF copies alternate DVE / ScalarE: one engine alone
                # (~540ns per [128, 384] copy) is slower than the MM cadence
                # and would gate phase A.
                dst = ys[:, j * R * c3:(j + 1) * R * c3]
                if j % 2 == 0:
                    nc.vector.tensor_copy(dst, pa[:])
                else:
                    nc.scalar.activation(dst, pa[:],
                                         mybir.ActivationFunctionType.Copy)

            # Main: acc_m[:c, :] += Y_g[ktile].T @ LgT[ktile, m-chunk]
            accs = [mpsum.tile([P, MM_N], f32, tag=f"acc{m}", name=f"acc{m}")
                    for m in range(mc_cnt)]

            def mm_block(g, kt0, tcnt, tile_, m_major=False):
                # m_major: all m=0 matmuls first, then m=1 -- used for the
                # final sub-tile so acc0's epilogue overlaps acc1's matmuls.
                loop = ([(t, m) for m in range(mc_cnt) for t in range(tcnt)]
                        if m_major else
                        [(t, m) for t in range(tcnt) for m in range(mc_cnt)])
                for t, m in loop:
                    kt = kt0 + t
                    ycol = kt * c3 + g * c
                    lhsT = ys[:, ycol:ycol + c]
                    first = g == 0 and kt == 0
                    last = g == 2 and kt == kt_total - 1
                    m0 = m * MM_N
                    m1 = min(rows, m0 + MM_N)
                    nc.tensor.matmul(
                        accs[m][:c, :m1 - m0],
                        lhsT,
                        tile_[:, t * rows + m0:t * rows + m1],
                        start=first, stop=last,
                    )

            # Schedule-order floor: the Tile scheduler's cost model
            # mispredicts the phase-A PSUM-copy latency and interleaves
            # (dependency-blocked) main matmuls between phase-A groups on the
            # in-order PE queue, stalling it.  Flooring the main matmuls'
            # schedule time pins the frozen PE order to
            # [warm-up, all of phase A, main]; at runtime only real data
            # dependencies remain, so nothing actually waits for the floor.
            def mm_floored(*args, **kwargs):
                with tc.tile_wait_until(0.05):
                    mm_block(*args, **kwargs)

            if hoist:
                for u, (st, sub) in enumerate(head_subs):
                    mm_floored(0, u * sub, sub, st)
            for g in range(3):
                lv = lv0 if g == 0 else LT[g][:].rearrange(lt_pat, t=t_pack, p=P)
                i0 = 1 if (g == 0 and hoist) else 0
                for i in range(i0, ni):
                    tail = (g == 2 and i == ni - 1 and t_pack >= 8
                            and tail_split)
                    if not tail:
                        lt = lpool.tile([P, t_pack * rows], ldt, tag="lt",
                                        name="lt")
                        lt_dma(lt, lv[i], t_pack)
                        mm_floored(g, i * t_pack, t_pack, lt)
                    else:
                        # Final tile split into small sub-tiles so the
                        # end-of-kernel PE chain after the last DMA is short.
                        sub = t_pack // 4
                        lvi = lv[i].rearrange("p (u t) m -> u p t m", u=4)
                        for u in range(4):
                            st = lpool.tile([P, sub * rows], ldt, tag="lt",
                                            name="lt_tail")
                            lt_dma(st, lvi[u], sub)
                            mm_floored(g, i * t_pack + u * sub, sub, st,
                                       m_major=(u == 3))

            # Per-m-chunk epilogue: relu(acc+bias) then DMA each half on its
            # own ring.  Chunk 0 uses ScalarE, chunk 1 uses DVE so the two
            # activations run in parallel right as each PSUM group stops.
            outsb = opool.tile([c, rows], f32)
            for m in range(mc_cnt):
                m0 = m * MM_N
                m1 = min(rows, m0 + MM_N)
                if m % 2 == 0:
                    nc.scalar.activation(
                        outsb[:, m0:m1], accs[m][:c, :m1 - m0],
                        mybir.ActivationFunctionType.Relu, bias=bs[:, 0:1])
                else:
                    nc.vector.tensor_scalar(
                        outsb[:, m0:m1], accs[m][:c, :m1 - m0],
                        bs[:, 0:1], 0.0,
                        mybir.AluOpType.add, mybir.AluOpType.max)
                eng = nc.sync if m % 2 == 0 else nc.scalar
                eng.dma_start(OUT[:, m0:m1], outsb[:, m0:m1])

    nc.compile()
    return nc


def make_in_maps(X, L0, L1, L2, W0, W1, W2, bias, n_cores=N_CORES):
    X = np.ascontiguousarray(np.asarray(X, dtype=np.float32))
    l_np = {"fp8e3": ml_dtypes.float8_e3m4, "fp16": np.float16,
            "f32r": np.float32, "fp32": np.float32}[L_DTYPE]
    Ls = [np.asarray(L, dtype=np.float32) for L in (L0, L1, L2)]
    Ws = [np.asarray(W, dtype=np.float32) for W in (W0, W1, W2)]
    bias = np.ascontiguousarray(np.asarray(bias, dtype=np.float32))

    fmm_np = (np.float16 if (USE_F32R and L_DTYPE != "fp32")
              else np.float32)
    n, c = X.shape
    XT = X.T  # [c, n]
    if CONTIG:
        # k-permutation mirroring the kernel's contiguous L.T reads: permuted
        # column chunk kt = i*T_PACK + t holds original k = i*T_PACK*128 +
        # T_PACK*p + t at position p.
        ni = n // (P * T_PACK)
        perm = (np.arange(ni)[:, None, None] * T_PACK * P
                + np.arange(T_PACK)[None, :, None]
                + T_PACK * np.arange(P)[None, None, :]).reshape(-1)
        XT = XT[:, perm]
    # Phase-A group q=(j, s) at xt4[32s:32s+32, j*128:(j+1)*128] must hold
    # the (permuted) k-range of main-loop k-tile kt = j*R + s:
    #   XT4[32s + c, j*128 + u] = XT[c, (j*R + s)*128 + u]
    kt_total = n // P
    XT4 = np.ascontiguousarray(
        XT.reshape(c, kt_total // R, R, P).transpose(2, 0, 1, 3)
        .reshape(P, n // R).astype(fmm_np))
    # Block-diagonal W stack: row-group s's c_in rows hit only column
    # group s:  Wstack[32s + ci, s*3c + g*c + cc] = Wg[ci, cc].
    Wstack = np.zeros((P, R * 3 * c), dtype=fmm_np)
    for s in range(R):
        for g in range(3):
            Wstack[32 * s:32 * (s + 1),
                   s * 3 * c + g * c:s * 3 * c + (g + 1) * c] = \
                Ws[g].astype(fmm_np)

    rows = n // n_cores
    in_maps = []
    for cid in range(n_cores):
        rc = slice(cid * rows, (cid + 1) * rows)
        m = {"XT4": XT4, "Wstack": Wstack, "bias": bias}
        for g in range(3):
            m[f"L{g}T"] = np.ascontiguousarray(Ls[g][rc].T.astype(l_np))
        in_maps.append(m)
    return in_maps


_NC_CACHE = {}


def _get_nc():
    key = (N, ROWS, T_PACK, LT_BUFS, USE_F32R, L_DTYPE, CONTIG, N_WARM)
    if key not in _NC_CACHE:
        _NC_CACHE[key] = build_nc(use_f32r=USE_F32R, l_dtype=L_DTYPE,
                                  contig=CONTIG, n_warm=N_WARM)
    return _NC_CACHE[key]


def run(inputs, trace=False, **kwargs):
    nc = _get_nc()
    in_maps = make_in_maps(**inputs)
    res = run_bass_kernel_spmd(nc, in_maps, core_ids=list(range(N_CORES)),
                               trace=trace, **kwargs)
    rows = N // N_CORES
    out = np.empty((N, C), dtype=np.float32)
    for cid in range(N_CORES):
        out[cid * rows:(cid + 1) * rows] = res.results[cid]["out"].T
    return out, res


def kernel(**inputs):
    out, _ = run(inputs, trace=False)
    return out

